# revision 2
# baseline (speedup 1.0000x reference)
"""Single-head causal attention (B=8, T=2048, D=1024, H=64) on 8 trn2 cores.

Data-parallel over batch: core b computes attention for x[b].

Numerics: scores = (x@Wq)(x@Wk)^T * sqrt(T) have std ~360 -> softmax is a
near-argmax; score errors must stay well below the minimum top-2 gap
(~4e-3).  All matmuls run in fp16 with fp32 PSUM accumulation, using hi/lo
splitting (x = xh + xl, W = Wh + Wl, products of fp16 are exact in fp32)
so projections and scores are fp32-accurate to ~1e-4.  Host pre-scales
x by 8 and W by 32 so the lo planes stay in fp16 normal range; the scale
is folded into the exp scale (sqrt(T)/2^16) and output normalization.
"""

import math

import numpy as np

import concourse.bass as bass
import concourse.mybir as mybir
import concourse.tile as tile
from concourse import bacc
from concourse.bass import ts
from concourse.bass_utils import run_bass_kernel_spmd
from concourse.masks import make_causal_mask, make_identity

B, T, D, H = 8, 2048, 1024, 64
NB = T // 128          # 16 query/key blocks of 128
DC = D // 128          # 8 contraction chunks
X_SCALE = 8.0
W_SCALE = 32.0
QK_SCALE = X_SCALE * X_SCALE * W_SCALE * W_SCALE      # q'*k' = 65536 * q*k
EXP_SCALE = float(np.float32(math.sqrt(T)) / QK_SCALE)
V_SCALE = X_SCALE * W_SCALE                           # v' = 256 * v
MASK_VAL = -1e9
FP16 = mybir.dt.float16
FP32 = mybir.dt.float32


def _chunks(L):
    """Split row-length L into matmul chunks of <=512 columns."""
    out = []
    c0 = 0
    while c0 < L:
        out.append((c0, min(512, L - c0)))
        c0 += 512
    return out


def build_program():
    nc = bacc.Bacc("TRN2", target_bir_lowering=False, debug=False)

    xh_d = nc.dram_tensor("xh", [T, D], FP16, kind="ExternalInput").ap()
    xl_d = nc.dram_tensor("xl", [T, D], FP16, kind="ExternalInput").ap()
    whi_d = nc.dram_tensor("whi", [D, 128], FP16, kind="ExternalInput").ap()
    wlo_d = nc.dram_tensor("wlo", [D, 128], FP16, kind="ExternalInput").ap()
    wvh_d = nc.dram_tensor("wvh", [D, H], FP16, kind="ExternalInput").ap()
    out_d = nc.dram_tensor("out", [T, H], FP32, kind="ExternalOutput").ap()

    with tile.TileContext(nc) as tc:
        with (
            tc.tile_pool(name="persist", bufs=1) as pp,
            tc.tile_pool(name="a_pool", bufs=2) as ap_,
            tc.tile_pool(name="at_pool", bufs=2) as atp,
            tc.tile_pool(name="small", bufs=3) as sp,
            tc.tile_pool(name="ps_big", bufs=4, space="PSUM") as psb,
            tc.tile_pool(name="ps_t", bufs=2, space="PSUM") as pst,
            tc.tile_pool(name="ps_o", bufs=2, space="PSUM") as pso,
        ):
            # ---- constants
            ident = pp.tile([128, 128], FP16)
            make_identity(nc, ident[:])
            cmask = pp.tile([128, 128], FP32)
            make_causal_mask(nc, cmask[:], mask_val=MASK_VAL)

            # ---- load weights: [D, M] -> [128, DC, M]
            whi = pp.tile([128, DC, 128], FP16)
            wlo = pp.tile([128, DC, 128], FP16)
            wvh = pp.tile([128, DC, H], FP16)
            nc.sync.dma_start(whi[:], whi_d.rearrange("(c p) m -> p c m", p=128))
            nc.sync.dma_start(wlo[:], wlo_d.rearrange("(c p) m -> p c m", p=128))
            nc.sync.dma_start(wvh[:], wvh_d.rearrange("(c p) m -> p c m", p=128))

            # ---- transposed x planes: [128, DC, T] fp16
            xth = pp.tile([128, DC, T], FP16)
            xtl = pp.tile([128, DC, T], FP16)
            for d in range(DC):
                nc.sync.dma_start_transpose(xth[:, d, :], xh_d[:, ts(d, 128)])
                nc.sync.dma_start_transpose(xtl[:, d, :], xl_d[:, ts(d, 128)])

            # ---- QK projections -> qhl [qh;ql], khh [kh;kh], kl
            qhl = pp.tile([128, T], FP16)     # rows 0:64 qh, 64:128 ql
            ql_tmp = pp.tile([64, T], FP16)
            khh = pp.tile([128, T], FP16)     # kh duplicated in both halves
            kl = pp.tile([128, T], FP16)      # rows 64:128 computed, 0:64 shifted
            for tcn in range(T // 512):
                pq = psb.tile([128, 512], FP32, tag="big")
                for d in range(DC):
                    first = d == 0
                    nc.tensor.matmul(pq[:], whi[:, d, :], xth[:, d, ts(tcn, 512)],
                                     start=first, stop=False)
                    nc.tensor.matmul(pq[:], whi[:, d, :], xtl[:, d, ts(tcn, 512)],
                                     start=False, stop=False)
                    nc.tensor.matmul(pq[:], wlo[:, d, :], xth[:, d, ts(tcn, 512)],
                                     start=False, stop=d == DC - 1)
                cs = ts(tcn, 512)
                # q half (rows 0:64)
                nc.scalar.copy(qhl[0:64, cs], pq[0:64, :])
                nc.vector.tensor_tensor(ql_tmp[:, cs], pq[0:64, :], qhl[0:64, cs],
                                        mybir.AluOpType.subtract)
                # k half (rows 64:128)
                nc.scalar.copy(khh[64:128, cs], pq[64:128, :])
                nc.vector.tensor_tensor(kl[64:128, cs], pq[64:128, :], khh[64:128, cs],
                                        mybir.AluOpType.subtract)
            # partition shifts (SBUF->SBUF DMA across partitions)
            nc.sync.dma_start(qhl[64:128, :], ql_tmp[:, :])
            nc.sync.dma_start(khh[0:64, :], khh[64:128, :])
            nc.sync.dma_start(kl[0:64, :], kl[64:128, :])

            # ---- V projection: vT [64, T] = Wvh^T x'h^T, then transpose to v
            vt = pp.tile([64, T], FP16)
            for tcn in range(T // 512):
                pv = psb.tile([128, 512], FP32, tag="big")
                for d in range(DC):
                    nc.tensor.matmul(pv[0:64, :], wvh[:, d, :],
                                     xth[:, d, ts(tcn, 512)],
                                     start=d == 0, stop=d == DC - 1)
                nc.scalar.copy(vt[:, ts(tcn, 512)], pv[0:64, :])
            v_sb = pp.tile([128, NB, H], FP16)
            for tb in range(NB):
                pvt = pst.tile([128, 128], FP16, tag="tr")
                nc.tensor.transpose(pvt[:, 0:64], vt[:, ts(tb, 128)], ident[0:64, 0:64])
                nc.vector.tensor_copy(v_sb[:, tb, :], pvt[:, 0:64])

            # ---- attention row-blocks
            r_all = pp.tile([128, NB], FP32)
            for i in range(NB):
                L = (i + 1) * 128
                chunks = _chunks(L)
                nch = len(chunks)
                a_sb = ap_.tile([128, 2048], FP16, tag="a")
                stats_m = sp.tile([128, 4], FP32, tag="sm")
                stats_s = sp.tile([128, 4], FP32, tag="ss")
                ps_chunks = []
                for ci, (c0, cw) in enumerate(chunks):
                    ps = psb.tile([128, 512], FP32, tag="big")
                    ps_chunks.append((ps, c0, cw))
                    nc.tensor.matmul(ps[:, 0:cw], qhl[:, ts(i, 128)],
                                     khh[:, c0:c0 + cw], start=True, stop=False)
                    nc.tensor.matmul(ps[:, 0:cw], qhl[0:64, ts(i, 128)],
                                     kl[0:64, c0:c0 + cw], start=False, stop=True)
                    if c0 + cw == L:  # diagonal block: apply causal mask
                        nc.vector.tensor_tensor(ps[:, cw - 128:cw], ps[:, cw - 128:cw],
                                                cmask[:], mybir.AluOpType.add)
                    nc.vector.tensor_reduce(stats_m[:, ci:ci + 1], ps[:, 0:cw],
                                            axis=mybir.AxisListType.X,
                                            op=mybir.AluOpType.max)
                m_i = sp.tile([128, 1], FP32, tag="mi")
                nc.vector.tensor_reduce(m_i[:], stats_m[:, 0:nch],
                                        axis=mybir.AxisListType.X,
                                        op=mybir.AluOpType.max)
                negb = sp.tile([128, 1], FP32, tag="nb")
                nc.vector.tensor_scalar_mul(negb[:], m_i[:], -EXP_SCALE)
                for ci, (ps, c0, cw) in enumerate(ps_chunks):
                    nc.scalar.activation(a_sb[:, c0:c0 + cw], ps[:, 0:cw],
                                         mybir.ActivationFunctionType.Exp,
                                         bias=negb[:], scale=EXP_SCALE,
                                         accum_out=stats_s[:, ci:ci + 1])
                rs = sp.tile([128, 1], FP32, tag="rs")
                nc.vector.tensor_reduce(rs[:], stats_s[:, 0:nch],
                                        axis=mybir.AxisListType.X,
                                        op=mybir.AluOpType.add)
                rinv = sp.tile([128, 1], FP32, tag="ri")
                nc.vector.reciprocal(rinv[:], rs[:])
                nc.vector.tensor_scalar_mul(r_all[:, i:i + 1], rinv[:], 1.0 / V_SCALE)

                # transpose A tiles, then AV
                at_sb = atp.tile([128, 2048], FP16, tag="at")
                for j in range(i + 1):
                    pt = pst.tile([128, 128], FP16, tag="tr")
                    nc.tensor.transpose(pt[:], a_sb[:, ts(j, 128)], ident[:])
                    if j % 2 == 0:
                        nc.vector.tensor_copy(at_sb[:, ts(j, 128)], pt[:])
                    else:
                        nc.scalar.copy(at_sb[:, ts(j, 128)], pt[:])
                po = pso.tile([128, H], FP32, tag="o")
                for j in range(i + 1):
                    nc.tensor.matmul(po[:], at_sb[:, ts(j, 128)], v_sb[:, j, :],
                                     start=j == 0, stop=j == i)
                o_sb = sp.tile([128, H], FP32, tag="ob")
                nc.vector.tensor_scalar_mul(o_sb[:], po[:], r_all[:, i:i + 1])
                nc.sync.dma_start(out_d[ts(i, 128), :], o_sb[:])

    nc.compile()
    return nc


_NC = None


def _get_nc():
    global _NC
    if _NC is None:
        _NC = build_program()
    return _NC


def _prep_core_inputs(xb, Wq, Wk, Wv):
    xs = (xb.astype(np.float32) * X_SCALE)
    xh = xs.astype(np.float16)
    xl = (xs - xh.astype(np.float32)).astype(np.float16)
    wqk = np.concatenate([Wq, Wk], axis=1).astype(np.float32) * W_SCALE
    whi = wqk.astype(np.float16)
    wlo = (wqk - whi.astype(np.float32)).astype(np.float16)
    wvh = (Wv.astype(np.float32) * W_SCALE).astype(np.float16)
    return {"xh": xh, "xl": xl, "whi": whi, "wlo": wlo, "wvh": wvh}


def kernel(x, Wq, Wk, Wv):
    x = np.asarray(x)
    Wq, Wk, Wv = np.asarray(Wq), np.asarray(Wk), np.asarray(Wv)
    nc = _get_nc()
    in_maps = [_prep_core_inputs(x[b], Wq, Wk, Wv) for b in range(B)]
    res = run_bass_kernel_spmd(nc, in_maps, core_ids=list(range(B)))
    return np.stack([res.results[b]["out"] for b in range(B)], axis=0)
